# revision 12
# baseline (speedup 1.0000x reference)
"""Trainium2 Bass kernel for nn_DotAtt_40097814675537.

Math (matches the reference up to fp error within the harness tolerance):
    score = Q @ K^T / sqrt(d)        [B, Sq, Sk]
    x     = score @ V                [B, Sq, dv]
    out   = softmax(where(j > valid_len[q], -1e6, x[b, q, j]), axis=-1)

Design:
  * Associativity: x = (Q / sqrt(d)) @ (K^T @ V) - contraction 2048 -> 512
    for the big matmul (exact math, only fp rounding differs).
  * Data-parallel over batch B=8, one batch per NeuronCore, no collectives.
  * Single-pass fp16 matmuls (inputs rounded to fp16, fp32 PSUM accumulate).
    The harness gate is rel_err < 2e-2; numpy simulation of this exact
    dataflow gives rel_err ~= 2.7e-3 (absmax ~= 2.4e-2), a 7x margin.
    This is 3x less tensor work and half the DMA bytes of an fp32-accurate
    hi/lo-split version.
  * Sorted-query specialization: the host sorts queries by valid_len (a row
    permutation - exact for row-wise softmax), so each 128-row tile only
    needs columns [0, max(valid_len)+1).  Widest tiles run first.
  * Mask built on-device from a DMA'd iota row + per-row valid_len
    ((iota > vl) * -60000 via one tensor_scalar op per tile) - no 1MB mask
    transfer.
  * Per tile: fused (x + mask, -max) via tensor_tensor_reduce, then
    exp(x - max) on ScalarE straight to fp16.  Unnormalized exp is stored;
    the host divides by the row sum (exact same softmax value).
  * Outputs packed [128, sum_w] fp16 and stored in 4 grouped DMAs so the
    tail isn't serialized on per-tile descriptor generation.
"""

import math
import sys
import types

import numpy as np

B, SQ, SK, D, DV = 8, 2048, 2048, 512, 512
N_CORES = 8
P = 128  # partitions
SC = SK // P  # 16 s-chunks for the K^T V contraction
DC = D // P  # 4 d-chunks for the Q M contraction
QT_TILES = SQ // P  # 16 query row tiles
NEG_FILL = -60000.0  # fits fp16; exp(x + NEG_FILL - max) == 0 in fp32
NGROUP = 4  # output store grouping

_CACHE = {}


def _install_ntff_hook():
    """antenv.axon_hooks is absent in this image; provide it so trace=True
    profiling works when requested (used by test.py, harmless otherwise)."""
    if "antenv.axon_hooks" in sys.modules:
        return
    try:
        from trn_agent_boot.trn_boot import _ntff_profile_via_ctypes

        hook = _ntff_profile_via_ctypes("/opt/axon/libaxon_pjrt.so")
    except Exception:
        hook = None
    mod = types.ModuleType("antenv.axon_hooks")
    mod.get_axon_ntff_profile_hook = lambda: hook
    mod.set_axon_ntff_profile_hook = lambda h: None
    sys.modules["antenv.axon_hooks"] = mod


def _build(widths):
    import concourse.tile as tile
    from concourse import bacc, mybir

    nc = bacc.Bacc("TRN2", target_bir_lowering=False, debug=False, num_devices=N_CORES)
    f32 = mybir.dt.float32
    f16 = mybir.dt.float16
    bf16 = mybir.dt.bfloat16

    sum_w = sum(widths)
    offs = [0]
    for w in widths:
        offs.append(offs[-1] + w)

    # All inputs partition-major fp16 so each SBUF partition's data is one
    # contiguous DRAM run per block (K and V interleaved per s-chunk for
    # 2KB-per-chunk descriptor runs):
    #   kv:  [128, SC*1024] kv[p, s*1024 + :512] = K row s*128+p,
    #                       kv[p, s*1024 + 512:] = V row s*128+p
    #   qt:  [128, 16*512]  qt[p, pos*512 + c*128+i] = Qs[order[pos]*128+i, c*128+p]
    #   aux: [128, 528]     cols 0:512 iota row (same every p), 512+pos: vl
    KCOLS = SC * 2 * D  # interleaved [k_chunk | v_chunk] per s-chunk
    QCOLS = QT_TILES * D
    kv_d = nc.dram_tensor("kv", [P, KCOLS], f16, kind="ExternalInput")
    qt_d = nc.dram_tensor("qt", [P, QCOLS], f16, kind="ExternalInput")
    aux_d = nc.dram_tensor("aux", [P, DV + QT_TILES], f32, kind="ExternalInput")
    id_d = nc.dram_tensor("ident", [P, P], f16, kind="ExternalInput")
    o_d = nc.dram_tensor("o", [P, sum_w], f16, kind="ExternalOutput")

    with tile.TileContext(nc) as tc:
        with (
            tc.tile_pool(name="big", bufs=1) as big,
            tc.tile_pool(name="mprime", bufs=1) as mp_pool,
            tc.tile_pool(name="psm", bufs=1, space="PSUM") as psum_m,
            tc.tile_pool(name="psx", bufs=4, space="PSUM") as psum_x,
            tc.tile_pool(name="egrp", bufs=3) as egrp,
            tc.tile_pool(name="stats", bufs=8) as stats,
        ):
            kvt = big.tile([P, KCOLS], f16, tag="kv", name="kv_sb")
            qtt = big.tile([P, QCOLS], f16, tag="qt", name="qt_sb")
            aux_t = big.tile([P, DV + QT_TILES], f32, tag="aux", name="aux_sb")
            id_t = big.tile([P, P], f16, tag="ident", name="id_sb")
            mask_t = big.tile([P, sum_w], f16, tag="mask", name="mask_sb")

            # Input loads in priority order (geometric K/V blocks so the
            # first matmul starts early while later blocks amortize issue
            # cost), then aux, then Q^T in consumption (pos) order.
            CHUNK = 2 * DV  # columns per interleaved kv s-chunk
            # All loads serial on Sync in consumption order (DMA ring FIFOs
            # drain in issue order).  2-chunk kv blocks keep descriptor runs
            # at 4KB: per-queue DMA rate saturates there, while longer runs
            # hog SBUF write ports and stall the PE operand stream.
            pos_ = 0
            for i, nchunk in enumerate((1, 1, 2, 2, 2, 2, 2, 2, 2)):
                lo, hi = pos_ * CHUNK, (pos_ + nchunk) * CHUNK
                nc.sync.dma_start(out=kvt[:, lo:hi], in_=kv_d[:, lo:hi])
                pos_ += nchunk
                if i == 4:
                    nc.sync.dma_start(out=aux_t, in_=aux_d[:, :])
                    nc.sync.dma_start(out=id_t, in_=id_d[:, :])
            pos_ = 0
            for nblk in (4, 4, 4, 4):
                lo, hi = pos_ * D, (pos_ + nblk) * D
                nc.sync.dma_start(out=qtt[:, lo:hi], in_=qt_d[:, lo:hi])
                pos_ += nblk

            # Per-tile additive masks, built on DVE during phase 1:
            # mask[p, j] = (iota[j] > vl[p]) * -60000
            for t in range(QT_TILES):
                W = widths[t]
                nc.vector.tensor_scalar(
                    mask_t[:, offs[t] : offs[t] + W],
                    aux_t[:, 0:W],
                    aux_t[:, DV + t : DV + t + 1],
                    NEG_FILL,
                    mybir.AluOpType.is_gt,
                    mybir.AluOpType.mult,
                )

            # Phase 1: M = K^T V over 16 s-chunks, single fp16 pass
            psums = [
                psum_m.tile([P, DV], f32, tag=f"m{c}", name=f"psum_m{c}")
                for c in range(DC)
            ]
            for s in range(SC):
                base = s * CHUNK
                vh = kvt[:, base + DV : base + 2 * DV]
                for c in range(DC):
                    kh = kvt[:, base + c * P : base + (c + 1) * P]
                    nc.tensor.matmul(
                        psums[c][:, :], kh, vh, start=(s == 0), stop=(s == SC - 1)
                    )

            # M PSUM -> SBUF fp16 (DVE copy; scheduled right after each
            # psum's last accumulation)
            mhis = []
            for c in range(DC):
                mhi = mp_pool.tile([P, DV], f16, tag=f"mh{c}", name=f"mhi{c}")
                if c < 2:
                    nc.scalar.copy(mhi[:, :], psums[c][:, :])
                else:
                    nc.vector.tensor_scalar_add(mhi[:, :], psums[c][:, :], 0.0)
                mhis.append(mhi)

            # Phase 2: per query tile (width W): X = Qs M, mask, softmax.
            # exp(x - max) goes straight to fp16 into the group buffer; one
            # store per group of 4 tiles.
            groups = [
                list(range(lo, hi))
                for lo, hi in ((0, 4), (4, 8), (8, 10), (10, 12), (12, 14), (14, 16))
            ]
            gwmax = max(sum(widths[t] for t in tl) for tl in groups)
            for gi, tiles in enumerate(groups):
                gw = sum(widths[t] for t in tiles)
                ex = egrp.tile([P, gwmax], f16, tag="e")
                goff = 0
                for t in tiles:
                    W = widths[t]
                    px = psum_x.tile([P, DV], f32, tag="x")
                    for c in range(DC):
                        qh = qtt[:, t * D + c * P : t * D + (c + 1) * P]
                        nc.tensor.matmul(
                            px[:, 0:W],
                            qh,
                            mhis[c][:, 0:W],
                            start=(c == 0),
                            stop=False,
                        )
                    # mask folded into the accumulation: ident^T @ mask == mask
                    # (last, so the identity LDWEIGHTS hides under the c=3 stream)
                    nc.tensor.matmul(
                        px[:, 0:W],
                        id_t[:, :],
                        mask_t[:, offs[t] : offs[t] + W],
                        start=False,
                        stop=True,
                    )
                    nmx = stats.tile([P, 1], f32, tag="nmn")
                    nc.vector.tensor_reduce(
                        out=nmx,
                        in_=px[:, 0:W],
                        axis=mybir.AxisListType.X,
                        op=mybir.AluOpType.max,
                        negate=True,
                    )
                    # e = exp(x_masked - max), fp16, straight from PSUM
                    nc.scalar.activation(
                        ex[:, goff : goff + W],
                        px[:, 0:W],
                        mybir.ActivationFunctionType.Exp,
                        bias=nmx[:, :],
                        scale=1.0,
                    )
                    goff += W
                glo = offs[tiles[0]]
                # last group's store issues from the idle Scalar queue so the
                # final two store issues run in parallel, not serialized
                eng = nc.scalar if gi == len(groups) - 1 else nc.sync
                eng.dma_start(out=o_d[:, glo : glo + gw], in_=ex[:, 0:gw])

    nc.compile()
    return nc


def _get_nc(widths):
    key = tuple(widths)
    if key not in _CACHE:
        _install_ntff_hook()
        _CACHE[key] = _build(key)
    return _CACHE[key]


def kernel(K, V, Q, valid_len, _trace=False):
    from concourse.bass_utils import run_bass_kernel_spmd

    K = np.asarray(K, dtype=np.float32)
    V = np.asarray(V, dtype=np.float32)
    Q = np.asarray(Q, dtype=np.float32)
    vl = np.asarray(valid_len).astype(np.int64)

    # sort queries by valid_len (row permutation; exact for row-wise softmax)
    perm = np.argsort(vl, kind="stable")
    vls = vl[perm]
    tile_w = []
    for t in range(QT_TILES):
        w = int(vls[t * P : (t + 1) * P].max()) + 1
        tile_w.append(min(DV, -(-w // 2) * 2))  # even, exact otherwise

    # process widest tiles first; device/pos order is descending width
    order = sorted(range(QT_TILES), key=lambda i: tile_w[i], reverse=True)
    widths = tuple(tile_w[t] for t in order)
    offs = np.concatenate([[0], np.cumsum(widths)]).astype(np.int64)
    sum_w = int(offs[-1])

    # fp16 partition-major interleaved K|V per s-chunk:
    # kv[p, s*1024 + j] = K[s*128+p, j], kv[p, s*1024 + 512 + j] = V[s*128+p, j]
    K16 = K.astype(np.float16)
    V16 = V.astype(np.float16)

    def kv_pack(kb, vb):
        ks = kb.reshape(SC, P, DV)
        vs = vb.reshape(SC, P, DV)
        kv = np.stack([ks, vs], axis=2)  # [s, p, 2, 512]
        return np.ascontiguousarray(
            kv.transpose(1, 0, 2, 3).reshape(P, SC * 2 * DV)
        )

    # Q^T in pos order: qt[p, pos*512 + c*128 + i] = Qs[b, order[pos]*128+i, c*128+p]
    scale = np.float32(1.0 / math.sqrt(D))
    qp = (Q[:, perm, :] * scale).astype(np.float16)  # [B, SQ, D] sorted rows
    row_sel = np.concatenate(
        [np.arange(t * P, (t + 1) * P) for t in order]
    )  # device row pos*128+i -> sorted row index
    qsel = qp[:, row_sel, :]  # [B, SQ, D] in pos order
    # -> [B, P, 16*512]: for each pos block of 128 rows, transpose to [D, 128]
    qt16 = np.ascontiguousarray(
        qsel.reshape(B, QT_TILES, P, DC, P)  # [b, pos, i, c, p]
        .transpose(0, 4, 1, 3, 2)  # [b, p, pos, c, i]
        .reshape(B, P, QT_TILES * D)
    )

    # aux: iota row + per-pos valid_len columns (fp32: comparison scalars
    # must be fp32 on the DVE)
    aux = np.empty((P, DV + QT_TILES), dtype=np.float32)
    aux[:, 0:DV] = np.arange(DV, dtype=np.float32)[None, :]
    vls_pos = vls[row_sel].reshape(QT_TILES, P)  # [pos, i] -> rows of each pos
    aux[:, DV:] = vls_pos.T.astype(np.float32)

    nc = _get_nc(widths)
    ident = np.eye(P, dtype=np.float16)
    in_maps = [
        {
            "kv": kv_pack(K16[b], V16[b]),
            "qt": qt16[b],
            "aux": aux,
            "ident": ident,
        }
        for b in range(N_CORES)
    ]
    res = run_bass_kernel_spmd(
        nc, in_maps, core_ids=list(range(N_CORES)), trace=_trace
    )

    out = np.zeros((B, SQ, DV), dtype=np.float32)
    for b in range(N_CORES):
        e_pk = res.results[b]["o"]  # [128, sum_w] fp16
        for pos in range(QT_TILES):
            t = order[pos]
            W = widths[pos]
            e = e_pk[:, offs[pos] : offs[pos] + W].astype(np.float32)
            e /= e.sum(axis=-1, keepdims=True, dtype=np.float32)
            rows = perm[t * P : (t + 1) * P]
            out[b, rows, 0:W] = e
    if _trace:
        kernel.last_result = res
    return out


# revision 13
# speedup vs baseline: 1.0049x; 1.0049x over previous
"""Trainium2 Bass kernel for nn_DotAtt_40097814675537.

Math (matches the reference up to fp error within the harness tolerance):
    score = Q @ K^T / sqrt(d)        [B, Sq, Sk]
    x     = score @ V                [B, Sq, dv]
    out   = softmax(where(j > valid_len[q], -1e6, x[b, q, j]), axis=-1)

Design:
  * Associativity: x = (Q / sqrt(d)) @ (K^T @ V) - contraction 2048 -> 512
    for the big matmul (exact math, only fp rounding differs).
  * Data-parallel over batch B=8, one batch per NeuronCore, no collectives.
  * Single-pass fp16 matmuls (inputs rounded to fp16, fp32 PSUM accumulate).
    The harness gate is rel_err < 2e-2; numpy simulation of this exact
    dataflow gives rel_err ~= 2.7e-3 (absmax ~= 2.4e-2), a 7x margin.
    This is 3x less tensor work and half the DMA bytes of an fp32-accurate
    hi/lo-split version.
  * Sorted-query specialization: the host sorts queries by valid_len (a row
    permutation - exact for row-wise softmax), so each 128-row tile only
    needs columns [0, max(valid_len)+1).  Widest tiles run first.
  * Mask built on-device from a DMA'd iota row + per-row valid_len
    ((iota > vl) * -60000 via one tensor_scalar op per tile) - no 1MB mask
    transfer.
  * Mask applied on the TensorE via an identity-weight matmul
    (ident^T @ mask accumulates the additive mask straight into PSUM),
    so the DVE only does the row-max reduce and ScalarE computes
    exp(x - max) from PSUM directly to fp16.  Unnormalized exp is stored;
    the host divides by the row sum (exact same softmax value).
  * All loads issue serially on the Sync queue in consumption order with
    ~4KB-per-partition descriptor runs: big enough for full per-queue DMA
    rate, small enough not to stall the PE operand stream (16KB runs cost
    ~80% matmul slowdown during loads; 4KB ~0%).
  * Outputs packed [128, sum_w] fp16, stored in 6 grouped DMAs (finer at
    the tail; last group issued from the idle Scalar queue).
  * NOTE: nc.vector.tensor_tensor_reduce passes CoreSim but crashes real
    hardware (NRT_EXEC_UNIT_UNRECOVERABLE) - do not use it.
"""

import math
import sys
import types

import numpy as np

B, SQ, SK, D, DV = 8, 2048, 2048, 512, 512
N_CORES = 8
P = 128  # partitions
SC = SK // P  # 16 s-chunks for the K^T V contraction
DC = D // P  # 4 d-chunks for the Q M contraction
QT_TILES = SQ // P  # 16 query row tiles
NEG_FILL = -60000.0  # fits fp16; exp(x + NEG_FILL - max) == 0 in fp32
NGROUP = 4  # output store grouping

_CACHE = {}


def _install_ntff_hook():
    """antenv.axon_hooks is absent in this image; provide it so trace=True
    profiling works when requested (used by test.py, harmless otherwise)."""
    if "antenv.axon_hooks" in sys.modules:
        return
    try:
        from trn_agent_boot.trn_boot import _ntff_profile_via_ctypes

        hook = _ntff_profile_via_ctypes("/opt/axon/libaxon_pjrt.so")
    except Exception:
        hook = None
    mod = types.ModuleType("antenv.axon_hooks")
    mod.get_axon_ntff_profile_hook = lambda: hook
    mod.set_axon_ntff_profile_hook = lambda h: None
    sys.modules["antenv.axon_hooks"] = mod


def _build(widths):
    import concourse.tile as tile
    from concourse import bacc, mybir

    nc = bacc.Bacc("TRN2", target_bir_lowering=False, debug=False, num_devices=N_CORES)
    f32 = mybir.dt.float32
    f16 = mybir.dt.float16
    bf16 = mybir.dt.bfloat16

    sum_w = sum(widths)
    offs = [0]
    for w in widths:
        offs.append(offs[-1] + w)

    # All inputs partition-major fp16 so each SBUF partition's data is one
    # contiguous DRAM run per block (K and V interleaved per s-chunk for
    # 2KB-per-chunk descriptor runs):
    #   kv:  [128, SC*1024] kv[p, s*1024 + :512] = K row s*128+p,
    #                       kv[p, s*1024 + 512:] = V row s*128+p
    #   qt:  [128, 16*512]  qt[p, pos*512 + c*128+i] = Qs[order[pos]*128+i, c*128+p]
    #   aux: [128, 528]     cols 0:512 iota row (same every p), 512+pos: vl
    KCOLS = SC * 2 * D  # interleaved [k_chunk | v_chunk] per s-chunk
    QCOLS = QT_TILES * D
    kv_d = nc.dram_tensor("kv", [P, KCOLS], f16, kind="ExternalInput")
    qt_d = nc.dram_tensor("qt", [P, QCOLS], f16, kind="ExternalInput")
    aux_d = nc.dram_tensor("aux", [P, DV + QT_TILES], f32, kind="ExternalInput")
    id_d = nc.dram_tensor("ident", [P, P], f16, kind="ExternalInput")
    o_d = nc.dram_tensor("o", [P, sum_w], f16, kind="ExternalOutput")

    with tile.TileContext(nc) as tc:
        with (
            tc.tile_pool(name="big", bufs=1) as big,
            tc.tile_pool(name="mprime", bufs=1) as mp_pool,
            tc.tile_pool(name="psm", bufs=1, space="PSUM") as psum_m,
            tc.tile_pool(name="psx", bufs=4, space="PSUM") as psum_x,
            tc.tile_pool(name="egrp", bufs=3) as egrp,
            tc.tile_pool(name="stats", bufs=8) as stats,
        ):
            kvt = big.tile([P, KCOLS], f16, tag="kv", name="kv_sb")
            qtt = big.tile([P, QCOLS], f16, tag="qt", name="qt_sb")
            aux_t = big.tile([P, DV + QT_TILES], f32, tag="aux", name="aux_sb")
            id_t = big.tile([P, P], f16, tag="ident", name="id_sb")
            mask_t = big.tile([P, sum_w], f16, tag="mask", name="mask_sb")

            # Input loads in priority order (geometric K/V blocks so the
            # first matmul starts early while later blocks amortize issue
            # cost), then aux, then Q^T in consumption (pos) order.
            CHUNK = 2 * DV  # columns per interleaved kv s-chunk
            # All loads serial on Sync in consumption order (DMA ring FIFOs
            # drain in issue order).  2-chunk kv blocks keep descriptor runs
            # at 4KB: per-queue DMA rate saturates there, while longer runs
            # hog SBUF write ports and stall the PE operand stream.
            pos_ = 0
            for i, nchunk in enumerate((1, 1, 2, 2, 2, 2, 2, 2, 2)):
                lo, hi = pos_ * CHUNK, (pos_ + nchunk) * CHUNK
                nc.sync.dma_start(out=kvt[:, lo:hi], in_=kv_d[:, lo:hi])
                pos_ += nchunk
                if i == 4:
                    nc.sync.dma_start(out=aux_t, in_=aux_d[:, :])
                    nc.sync.dma_start(out=id_t, in_=id_d[:, :])
            pos_ = 0
            for nblk in (4, 4, 4, 4):
                lo, hi = pos_ * D, (pos_ + nblk) * D
                nc.sync.dma_start(out=qtt[:, lo:hi], in_=qt_d[:, lo:hi])
                pos_ += nblk

            # Per-tile additive masks, built on DVE during phase 1:
            # mask[p, j] = (iota[j] > vl[p]) * -60000
            for t in range(QT_TILES):
                W = widths[t]
                nc.vector.tensor_scalar(
                    mask_t[:, offs[t] : offs[t] + W],
                    aux_t[:, 0:W],
                    aux_t[:, DV + t : DV + t + 1],
                    NEG_FILL,
                    mybir.AluOpType.is_gt,
                    mybir.AluOpType.mult,
                )

            # Phase 1: M = K^T V over 16 s-chunks, single fp16 pass
            psums = [
                psum_m.tile([P, DV], f32, tag=f"m{c}", name=f"psum_m{c}")
                for c in range(DC)
            ]
            for s in range(SC):
                base = s * CHUNK
                vh = kvt[:, base + DV : base + 2 * DV]
                for c in range(DC):
                    kh = kvt[:, base + c * P : base + (c + 1) * P]
                    nc.tensor.matmul(
                        psums[c][:, :], kh, vh, start=(s == 0), stop=(s == SC - 1)
                    )

            # M PSUM -> SBUF fp16 (DVE copy; scheduled right after each
            # psum's last accumulation)
            mhis = []
            for c in range(DC):
                mhi = mp_pool.tile([P, DV], f16, tag=f"mh{c}", name=f"mhi{c}")
                if c < 2:
                    nc.scalar.copy(mhi[:, :], psums[c][:, :])
                else:
                    nc.vector.tensor_scalar_add(mhi[:, :], psums[c][:, :], 0.0)
                mhis.append(mhi)

            # Phase 2: per query tile (width W): X = Qs M, mask, softmax.
            # exp(x - max) goes straight to fp16 into the group buffer; one
            # store per group of 4 tiles.
            groups = [
                list(range(lo, hi))
                for lo, hi in ((0, 4), (4, 8), (8, 10), (10, 12), (12, 14), (14, 16))
            ]
            gwmax = max(sum(widths[t] for t in tl) for tl in groups)
            for gi, tiles in enumerate(groups):
                gw = sum(widths[t] for t in tiles)
                ex = egrp.tile([P, gwmax], f16, tag="e")
                goff = 0
                for t in tiles:
                    W = widths[t]
                    px = psum_x.tile([P, DV], f32, tag="x")
                    for c in range(DC):
                        qh = qtt[:, t * D + c * P : t * D + (c + 1) * P]
                        nc.tensor.matmul(
                            px[:, 0:W],
                            qh,
                            mhis[c][:, 0:W],
                            start=(c == 0),
                            stop=False,
                        )
                    # mask folded into the accumulation: ident^T @ mask == mask
                    # (last, so the identity LDWEIGHTS hides under the c=3 stream)
                    nc.tensor.matmul(
                        px[:, 0:W],
                        id_t[:, :],
                        mask_t[:, offs[t] : offs[t] + W],
                        start=False,
                        stop=True,
                    )
                    nmx = stats.tile([P, 1], f32, tag="nmn")
                    nc.vector.tensor_reduce(
                        out=nmx,
                        in_=px[:, 0:W],
                        axis=mybir.AxisListType.X,
                        op=mybir.AluOpType.max,
                        negate=True,
                    )
                    # e = exp(x_masked - max), fp16, straight from PSUM
                    nc.scalar.activation(
                        ex[:, goff : goff + W],
                        px[:, 0:W],
                        mybir.ActivationFunctionType.Exp,
                        bias=nmx[:, :],
                        scale=1.0,
                    )
                    goff += W
                glo = offs[tiles[0]]
                # last group's store issues from the idle Scalar queue so the
                # final two store issues run in parallel, not serialized
                eng = nc.scalar if gi == len(groups) - 1 else nc.sync
                eng.dma_start(out=o_d[:, glo : glo + gw], in_=ex[:, 0:gw])

    nc.compile()
    return nc


def _get_nc(widths):
    key = tuple(widths)
    if key not in _CACHE:
        _install_ntff_hook()
        _CACHE[key] = _build(key)
    return _CACHE[key]


def kernel(K, V, Q, valid_len, _trace=False):
    from concourse.bass_utils import run_bass_kernel_spmd

    K = np.asarray(K, dtype=np.float32)
    V = np.asarray(V, dtype=np.float32)
    Q = np.asarray(Q, dtype=np.float32)
    vl = np.asarray(valid_len).astype(np.int64)

    # sort queries by valid_len (row permutation; exact for row-wise softmax)
    perm = np.argsort(vl, kind="stable")
    vls = vl[perm]
    tile_w = []
    for t in range(QT_TILES):
        w = int(vls[t * P : (t + 1) * P].max()) + 1
        tile_w.append(min(DV, -(-w // 2) * 2))  # even, exact otherwise

    # process widest tiles first; device/pos order is descending width
    order = sorted(range(QT_TILES), key=lambda i: tile_w[i], reverse=True)
    widths = tuple(tile_w[t] for t in order)
    offs = np.concatenate([[0], np.cumsum(widths)]).astype(np.int64)
    sum_w = int(offs[-1])

    # fp16 partition-major interleaved K|V per s-chunk:
    # kv[p, s*1024 + j] = K[s*128+p, j], kv[p, s*1024 + 512 + j] = V[s*128+p, j]
    K16 = K.astype(np.float16)
    V16 = V.astype(np.float16)

    def kv_pack(kb, vb):
        ks = kb.reshape(SC, P, DV)
        vs = vb.reshape(SC, P, DV)
        kv = np.stack([ks, vs], axis=2)  # [s, p, 2, 512]
        return np.ascontiguousarray(
            kv.transpose(1, 0, 2, 3).reshape(P, SC * 2 * DV)
        )

    # Q^T in pos order: qt[p, pos*512 + c*128 + i] = Qs[b, order[pos]*128+i, c*128+p]
    scale = np.float32(1.0 / math.sqrt(D))
    qp = (Q[:, perm, :] * scale).astype(np.float16)  # [B, SQ, D] sorted rows
    row_sel = np.concatenate(
        [np.arange(t * P, (t + 1) * P) for t in order]
    )  # device row pos*128+i -> sorted row index
    qsel = qp[:, row_sel, :]  # [B, SQ, D] in pos order
    # -> [B, P, 16*512]: for each pos block of 128 rows, transpose to [D, 128]
    qt16 = np.ascontiguousarray(
        qsel.reshape(B, QT_TILES, P, DC, P)  # [b, pos, i, c, p]
        .transpose(0, 4, 1, 3, 2)  # [b, p, pos, c, i]
        .reshape(B, P, QT_TILES * D)
    )

    # aux: iota row + per-pos valid_len columns (fp32: comparison scalars
    # must be fp32 on the DVE)
    aux = np.empty((P, DV + QT_TILES), dtype=np.float32)
    aux[:, 0:DV] = np.arange(DV, dtype=np.float32)[None, :]
    vls_pos = vls[row_sel].reshape(QT_TILES, P)  # [pos, i] -> rows of each pos
    aux[:, DV:] = vls_pos.T.astype(np.float32)

    nc = _get_nc(widths)
    ident = np.eye(P, dtype=np.float16)
    in_maps = [
        {
            "kv": kv_pack(K16[b], V16[b]),
            "qt": qt16[b],
            "aux": aux,
            "ident": ident,
        }
        for b in range(N_CORES)
    ]
    res = run_bass_kernel_spmd(
        nc, in_maps, core_ids=list(range(N_CORES)), trace=_trace
    )

    out = np.zeros((B, SQ, DV), dtype=np.float32)
    for b in range(N_CORES):
        e_pk = res.results[b]["o"]  # [128, sum_w] fp16
        for pos in range(QT_TILES):
            t = order[pos]
            W = widths[pos]
            e = e_pk[:, offs[pos] : offs[pos] + W].astype(np.float32)
            e /= e.sum(axis=-1, keepdims=True, dtype=np.float32)
            rows = perm[t * P : (t + 1) * P]
            out[b, rows, 0:W] = e
    if _trace:
        kernel.last_result = res
    return out
